# revision 1
# baseline (speedup 1.0000x reference)
"""Self-attention (channel attention) kernel for Trainium2, 8-core SPMD.

Problem: x (2,16,16,16,64) fp32 -> q = x.reshape(B=2, N=4096, C=64)
  energy = q @ q^T  (per batch, N x N)
  attn = softmax(energy, axis=-1)
  out = gamma * (attn @ q) + x

Sharding: each of the 8 cores computes 512 q-rows of BOTH batches
(core c handles rows [512c, 512c+512)). Each core receives the full x
as keys (pre-cast to bf16 on the host, halving HBM traffic) plus its
fp32 q-slice for the residual, and returns its (2, 512, 64) slab.

v4 structure (PE cycles/chunk: S 512 row-tiled + PV 1024 + tr 128):
  - 12 warm-up matmuls on junk SBUF emitted as the FIRST PE
    instructions (no DMA deps) so the clock ramps during the preamble
  - kt built with +8 chunk lookahead; 4 transposes share one psum tile
    and ONE DVE copy (amortizes the psum access penalty)
  - exp split 20 ACT / 12 DVE: ACT chunks Exp(s-64)->bf16; DVE chunks
    sat_u16(s*184.665 + 4437.3) = Schraudolph bf16 bits (fp32->uint16
    saturates negatives to +0; softmax renormalization hides the ~2%
    weight error - verified numerically and on HW)
  - PV pair [K|1]^T @ P^T, fp32 psum accumulate, LAG=3 behind exp
  - bf16 utility copies (kbf65, xq_bf) on GpSimd; PV-psum drains on ACT
  - all DMA triggers on the sync ring; epilogue ends in one output DMA
    per batch
"""

import sys

try:
    import concourse  # noqa: F401
except ImportError:
    sys.path.insert(0, "/opt/trn_rl_repo")

import numpy as np

N_CORES = 8
B = 2
N = 4096
C = 64
QROWS = N // N_CORES        # 512 q rows per core (per batch)
NT = N // 128               # 32 key tiles
QT_TILES = QROWS // 128     # 4 q tiles

DVE_CHUNKS = frozenset(j for j in range(NT) if j % 8 in (2, 4, 7))  # 12
LOOKAHEAD = 8
TRB = 4                     # transposes batched per psum tile / DVE copy
LAG = 3                     # chunks PV trails the exp stage by

LOG2E_128 = 184.6650390625                 # 128 * log2(e)
SCHRAU_BIAS = 16256.0 - 64.0 * LOG2E_128   # +4437.3...

_CACHE = {}


def _build_program():
    import concourse.bacc as bacc
    import concourse.tile as tile
    from concourse import mybir

    F32 = mybir.dt.float32
    BF16 = mybir.dt.bfloat16
    U16 = mybir.dt.uint16
    EXP = mybir.ActivationFunctionType.Exp
    COPY = mybir.ActivationFunctionType.Copy
    MULT = mybir.AluOpType.mult
    ADD = mybir.AluOpType.add

    nc = bacc.Bacc("TRN2", target_bir_lowering=False, debug=False)

    xkb_dram = nc.dram_tensor("xkb", [B, N, C], BF16, kind="ExternalInput")
    xq_dram = nc.dram_tensor("xq", [B, QROWS, C], F32, kind="ExternalInput")
    gam_dram = nc.dram_tensor("gam", [128, 1], F32, kind="ExternalInput")
    ident_dram = nc.dram_tensor("ident", [128, 128], F32, kind="ExternalInput")
    out_dram = nc.dram_tensor("out", [B, QROWS, C], F32, kind="ExternalOutput")

    with tile.TileContext(nc) as tc:
        with (
            tc.tile_pool(name="singles", bufs=1) as singles,
            tc.tile_pool(name="ptp", bufs=6) as ptp,
            tc.tile_pool(name="misc", bufs=8) as misc,
            tc.tile_pool(name="spsum", bufs=2, space="PSUM") as spsum,
            tc.tile_pool(name="trpsum", bufs=2, space="PSUM") as trpsum,
            tc.tile_pool(name="pvpsum", bufs=1, space="PSUM") as pvpsum,
        ):
            # warm-up seeds: junk SBUF contents are fine, the results land
            # in psum that is later overwritten with start=True
            junk = singles.tile([128, 128], BF16)
            nc.gpsimd.memset(junk[:], 1.0)
            pv_psA = pvpsum.tile([128, QROWS], F32, tag="pva")
            pv_psB = pvpsum.tile([128, QROWS], F32, tag="pvb")
            pv_ps = [pv_psA, pv_psB]
            for w in range(12):
                nc.tensor.matmul(
                    pv_ps[w % 2][:, 0:256], junk[:],
                    junk[:, None, 0:128].to_broadcast([128, 2, 128]),
                    start=True, stop=True,
                )

            ident = singles.tile([128, 128], F32)
            gam = singles.tile([128, 1], F32)
            neg64 = singles.tile([128, 1], F32)
            warm = singles.tile([128, 1], F32)
            xq_nat = singles.tile([128, QT_TILES, B, C], F32)
            knat_bf = singles.tile([128, NT, B, C], BF16)
            # PV stationary padded to 128 columns: col 64 = ones (row sums),
            # cols 65..127 = ones (junk rows in psum, never read) so the
            # full-128-col weight load takes the FWL fast path
            kbf65 = singles.tile([128, NT, B, 128], BF16)
            kt = singles.tile([128, NT, 128], BF16)
            qt = singles.tile([128, QROWS], BF16)
            ident_bf = singles.tile([128, 128], BF16)
            xq_bf = singles.tile([128, QT_TILES, B, C], BF16)
            obuf = singles.tile([128, B, QT_TILES, C], F32)

            GRP = 8  # key tiles per DMA trigger (per batch)
            NGRP = NT // GRP

            def dma_group(g, lo=0):
                rows = slice(128 * (GRP * g + lo), 128 * GRP * (g + 1))
                for b in range(B):
                    nc.sync.dma_start(
                        out=knat_bf[:, GRP * g + lo : GRP * (g + 1), b, :],
                        in_=xkb_dram.ap()[b, rows, :].rearrange(
                            "(t p) c -> p t c", p=128
                        ),
                    )

            def cast_group(g, lo=0):
                # [K | ones] PV stationary K-part, bf16->bf16 on GpSimd
                sl = slice(GRP * g + lo, GRP * (g + 1))
                nc.gpsimd.tensor_copy(
                    kbf65[:, sl, :, 0:C], knat_bf[:, sl, :, :]
                )

            def build_kt(j0, n=TRB):
                # n chunks -> one psum tile -> one DVE copy
                n = min(n, NT - j0)
                tr = trpsum.tile([128, TRB, 128], BF16, tag="trb")
                for i in range(n):
                    nc.tensor.transpose(
                        tr[:, i, :], knat_bf[:, j0 + i, :, :], ident_bf[:]
                    )
                nc.vector.tensor_copy(
                    kt[:, j0 : j0 + n, :], tr[:, 0:n, :]
                )

            # exp table preload + constants (no DMA deps)
            nc.vector.memset(warm[:], 0.0)
            nc.scalar.activation(warm[:], warm[:], EXP)
            nc.vector.memset(neg64[:], -64.0)
            ones_bf = singles.tile([128, 1], BF16)
            nc.vector.memset(ones_bf[:], 1.0)
            nc.vector.tensor_copy(
                kbf65[:, :, :, C:128],
                ones_bf[:, None, None, :].to_broadcast([128, NT, B, 128 - C]),
            )

            # DMAs on the sync ring: ident first (it gates every transpose),
            # then the q-slice (qt gates the loop), key chunk 0 alone, then
            # key groups; gam is only needed in the epilogue so it goes last
            nc.sync.dma_start(out=ident[:], in_=ident_dram.ap())
            nc.sync.dma_start(
                out=xq_nat[:, :, 0, :],
                in_=xq_dram.ap()[0].rearrange("(t p) c -> p t c", p=128),
            )
            nc.sync.dma_start(
                out=xq_nat[:, :, 1, :],
                in_=xq_dram.ap()[1].rearrange("(t p) c -> p t c", p=128),
            )
            nc.sync.dma_start(
                out=knat_bf[:, 0:1, 0, :],
                in_=xkb_dram.ap()[0, 0:128, :][None].rearrange("o p c -> p o c"),
            )
            nc.sync.dma_start(
                out=knat_bf[:, 0:1, 1, :],
                in_=xkb_dram.ap()[1, 0:128, :][None].rearrange("o p c -> p o c"),
            )
            dma_group(0, lo=1)
            dma_group(1)

            nc.vector.tensor_copy(ident_bf[:], ident[:])
            nc.gpsimd.tensor_copy(xq_bf[:], xq_nat[:])

            # Q^T via one [128,128] PE transpose per q tile (batched copy)
            trq = trpsum.tile([128, TRB, 128], BF16, tag="trb")
            for t in range(QT_TILES):
                nc.tensor.transpose(trq[:, t, :], xq_bf[:, t, :, :], ident_bf[:])
            nc.vector.tensor_copy(
                qt[:].rearrange("p (t x) -> p t x", t=QT_TILES), trq[:]
            )

            cast_group(0)
            # chunk 0's kt comes from its own early DMA; don't batch it with
            # chunks 1-3 (those wait on the full first key group)
            build_kt(0, n=1)
            build_kt(1, n=3)
            build_kt(4, n=4)
            dma_group(2)
            cast_group(1)
            nc.sync.dma_start(out=gam[:], in_=gam_dram.ap())

            # software-pipelined main loop
            pt_q = {}
            for j in range(NT + LAG):
                if j < NT:
                    if j % GRP == 0:
                        if j // GRP + 3 < NGRP:
                            dma_group(j // GRP + 3)
                        if j // GRP + 2 < NGRP:
                            cast_group(j // GRP + 2)
                    if (j + LOOKAHEAD) < NT and (j + LOOKAHEAD) % TRB == 0:
                        build_kt(j + LOOKAHEAD)

                    # S chunk: [128 keys, b, 512 qrows] fp32 psum
                    s_ps = spsum.tile([128, B, QROWS], F32, tag="s")
                    for b in range(B):
                        nc.tensor.matmul(
                            s_ps[:, b, :],
                            kt[64 * b : 64 * b + 64, j, :],
                            qt[64 * b : 64 * b + 64, :],
                            start=True,
                            stop=True,
                            tile_position=(64 * b, 0),
                        )

                    # P^T = exp(S^T - 64) -> bf16 (two engines)
                    pt_t = ptp.tile([128, B, QROWS], BF16, tag="pt")
                    if j in DVE_CHUNKS:
                        nc.vector.tensor_scalar(
                            pt_t[:].bitcast(U16), s_ps[:],
                            LOG2E_128, SCHRAU_BIAS, MULT, ADD,
                        )
                    else:
                        nc.scalar.activation(pt_t[:], s_ps[:], EXP, bias=neg64[:])
                    pt_q[j] = pt_t

                if j >= LAG:
                    jj = j - LAG
                    pt_prev = pt_q.pop(jj)
                    for b in range(B):
                        nc.tensor.matmul(
                            pv_ps[b][:, :],
                            kbf65[:, jj, b, :],
                            pt_prev[:, b, :],
                            start=(jj == 0),
                            stop=(jj == NT - 1),
                        )

            # ---- epilogue ----
            # pv_ps[b] rows 0..63 = O^T (unnormalized), row 64 = row sums.
            # psum->SBUF drain on ACT (free after the exp stream ends).
            ovs = {}
            for b in range(B):
                ovs[b] = singles.tile([C + 1, QROWS], F32, tag=f"ov{b}", name=f"ov{b}")
                nc.scalar.activation(ovs[b][:], pv_ps[b][0 : C + 1, :], COPY)
            for b in range(B):
                for t in range(QT_TILES):
                    cols = slice(128 * t, 128 * t + 128)
                    o_tr = spsum.tile([128, C + 1], F32, tag="s")
                    nc.tensor.transpose(
                        o_tr[:], ovs[b][:, cols], ident[0 : C + 1, 0 : C + 1]
                    )
                    recip = misc.tile([128, 1], F32, tag="recip")
                    nc.vector.reciprocal(recip[:], o_tr[:, C : C + 1])
                    scale = misc.tile([128, 1], F32, tag="scale")
                    nc.vector.tensor_tensor(scale[:], recip[:], gam[:], MULT)
                    nc.scalar.activation(
                        obuf[:, b, t, :], o_tr[:, 0:C], COPY, scale=scale[:]
                    )
                    nc.gpsimd.tensor_tensor(
                        obuf[:, b, t, :], obuf[:, b, t, :], xq_nat[:, t, b, :],
                        ADD,
                    )
                nc.sync.dma_start(
                    out=out_dram.ap()[b].rearrange("(t p) c -> p t c", p=128),
                    in_=obuf[:, b, :, :],
                )

    nc.compile()
    return nc


def _get_nc():
    if "nc" not in _CACHE:
        _CACHE["nc"] = _build_program()
    return _CACHE["nc"]


def kernel(x, gamma, _trace=False, _trace_kwargs=None):
    import ml_dtypes

    from concourse.bass_utils import run_bass_kernel_spmd

    x = np.asarray(x, dtype=np.float32)
    gamma = np.asarray(gamma, dtype=np.float32)
    shape_in = x.shape
    xk = np.ascontiguousarray(x.reshape(B, N, C))
    xkb = xk.astype(ml_dtypes.bfloat16)
    gam = np.full((128, 1), float(gamma.reshape(-1)[0]), dtype=np.float32)
    ident = np.eye(128, dtype=np.float32)

    nc = _get_nc()
    in_maps = [
        {
            "xkb": xkb,
            "xq": np.ascontiguousarray(xk[:, QROWS * c : QROWS * (c + 1), :]),
            "gam": gam,
            "ident": ident,
        }
        for c in range(N_CORES)
    ]
    res = run_bass_kernel_spmd(
        nc,
        in_maps,
        core_ids=list(range(N_CORES)),
        trace=_trace,
        **(_trace_kwargs or {}),
    )
    out = np.empty((B, N, C), dtype=np.float32)
    for c in range(N_CORES):
        out[:, QROWS * c : QROWS * (c + 1), :] = res.results[c]["out"]
    if _trace:
        _CACHE["last_results"] = res
    return out.reshape(shape_in)

